# revision 8
# baseline (speedup 1.0000x reference)
"""DeepPoly AbstractRelu elementwise kernel for Trainium2, SPMD over 8 NeuronCores.

Math (validated bit-level against the jax reference on the fixed dataset):
    x_out    = relu(x)
    high_out = relu(high)          (algebraically exact: the crossing-case chord
                                    evaluates to exactly `high`; <=3ulp fp diff)
    low_out  = low * [high > 0] * [-low <= high  (when high > 0)]
               -- the lambda condition low^2 > high^2 with low<0,high>0 is
                  exactly -low > high; comparing against `high` (== |high| when
                  high>0) is valid because the result is masked to 0 otherwise.

Sharding: pure elementwise, inputs split contiguously across 8 cores (dim 0).
"""

import numpy as np

N_TOTAL = 16777216
N_CORES = 8
N_CORE = N_TOTAL // N_CORES  # 2097152
P = 128
FD = 2048  # free-dim elements per tile -> [128, 2048] f32 = 1 MiB per DMA
NTILES = N_CORE // (P * FD)
BUFS = 3

_CACHE = {}


def _build_nc(repeat=1):
    from concourse import bacc, mybir
    from concourse.tile import TileContext

    f32 = mybir.dt.float32
    Alu = mybir.AluOpType
    Act = mybir.ActivationFunctionType

    # Bacc (not raw Bass): its compile() pass splits excess semaphore waits
    # onto InstEventSemaphore instructions (TRN2 allows 1 fused wait/inst).
    nc = bacc.Bacc(None, target_bir_lowering=False)
    x = nc.dram_tensor("x", [N_CORE], f32, kind="ExternalInput")
    low = nc.dram_tensor("low", [N_CORE], f32, kind="ExternalInput")
    high = nc.dram_tensor("high", [N_CORE], f32, kind="ExternalInput")
    x_out = nc.dram_tensor("x_out", [N_CORE], f32, kind="ExternalOutput")
    low_out = nc.dram_tensor("low_out", [N_CORE], f32, kind="ExternalOutput")
    high_out = nc.dram_tensor("high_out", [N_CORE], f32, kind="ExternalOutput")

    def tiled(t):
        return t.rearrange("(n p m) -> n p m", p=P, m=FD)

    xr, lr, hr = tiled(x), tiled(low), tiled(high)
    xor_, lor_, hor_ = tiled(x_out), tiled(low_out), tiled(high_out)

    with TileContext(nc) as tc:
        with tc.tile_pool(name="pool", bufs=BUFS) as pool:
            for i in [i for _ in range(repeat) for i in range(NTILES)]:
                xt = pool.tile([P, FD], f32)
                lt = pool.tile([P, FD], f32)
                ht = pool.tile([P, FD], f32)
                k1 = pool.tile([P, FD], f32)
                tt = pool.tile([P, FD], f32)

                nc.sync.dma_start(out=xt[:, :], in_=xr[i, :, :])
                nc.sync.dma_start(out=lt[:, :], in_=lr[i, :, :])
                nc.sync.dma_start(out=ht[:, :], in_=hr[i, :, :])

                # k1 = [(-low) <= high]  (lambda keep-mask; only consumed when high>0)
                nc.vector.scalar_tensor_tensor(
                    out=k1[:, :], in0=lt[:, :], scalar=-1.0, in1=ht[:, :],
                    op0=Alu.mult, op1=Alu.is_le,
                )
                # tt = [high > 0] * low  (zero the inactive case)
                nc.vector.scalar_tensor_tensor(
                    out=tt[:, :], in0=ht[:, :], scalar=0.0, in1=lt[:, :],
                    op0=Alu.is_gt, op1=Alu.mult,
                )
                # low_out = k1 * tt   (in-place into the low tile)
                nc.vector.tensor_mul(out=lt[:, :], in0=k1[:, :], in1=tt[:, :])

                # relus on the scalar/ACT engine, in place (after ht was read)
                nc.scalar.activation(out=xt[:, :], in_=xt[:, :], func=Act.Relu)
                nc.scalar.activation(out=ht[:, :], in_=ht[:, :], func=Act.Relu)

                nc.sync.dma_start(out=xor_[i, :, :], in_=xt[:, :])
                nc.sync.dma_start(out=lor_[i, :, :], in_=lt[:, :])
                nc.sync.dma_start(out=hor_[i, :, :], in_=ht[:, :])
    nc.finalize()  # runs Bacc.compile(): wait-splitting, reg alloc, DCE
    return nc


def _get_nc():
    if "nc" not in _CACHE:
        _CACHE["nc"] = _build_nc()
    return _CACHE["nc"]


def kernel(x, low, high, _trace=False):
    from concourse.bass_utils import run_bass_kernel_spmd

    x = np.ascontiguousarray(np.asarray(x, dtype=np.float32).reshape(-1))
    low = np.ascontiguousarray(np.asarray(low, dtype=np.float32).reshape(-1))
    high = np.ascontiguousarray(np.asarray(high, dtype=np.float32).reshape(-1))
    assert x.shape == (N_TOTAL,)

    in_maps = []
    for c in range(N_CORES):
        sl = slice(c * N_CORE, (c + 1) * N_CORE)
        in_maps.append({
            "x": np.ascontiguousarray(x[sl]),
            "low": np.ascontiguousarray(low[sl]),
            "high": np.ascontiguousarray(high[sl]),
        })

    nc = _get_nc()
    res = run_bass_kernel_spmd(
        nc, in_maps, core_ids=list(range(N_CORES)), trace=_trace,
    )
    if _trace:
        _CACHE["last_results"] = res

    x_out = np.concatenate([res.results[c]["x_out"] for c in range(N_CORES)])
    low_out = np.concatenate([res.results[c]["low_out"] for c in range(N_CORES)])
    high_out = np.concatenate([res.results[c]["high_out"] for c in range(N_CORES)])
    return np.stack([x_out, low_out, high_out])


# revision 28
# speedup vs baseline: 1.1746x; 1.1746x over previous
"""DeepPoly AbstractRelu elementwise kernel for Trainium2, SPMD over 8 NeuronCores.

Math (validated bit-level against the jax reference on the fixed dataset):
    x_out    = relu(x)
    high_out = relu(high)          (algebraically exact: the crossing-case chord
                                    ub_slope*high + ub_int evaluates to exactly
                                    `high`; <=3ulp fp difference vs reference)
    low_out  = low * [high > 0] * [(-low) <= high]
               -- reference lambda zeroes `low` when low<0, high>0 and
                  low^2 > high^2, which for low<0<high is exactly -low > high.
                  When low>=0 the comparison (-low) <= high is always true for
                  high>0, so one fused compare covers both branches; the
                  [high>0]*low factor zeroes every high<=0 case, making the
                  comparison's value irrelevant there (so comparing against
                  high rather than |high| is safe).

Sharding: pure elementwise over the neuron axis; inputs split contiguously
across the 8 cores (dim 0), no communication. Each core streams its 2M-element
slice through SBUF in [128, 2048] f32 tiles (1 MiB DMAs).

Engine layout (measured fastest): input DMAs issue from the SP HWDGE ring,
output DMAs + relus from the ACT ring (stores wait on compute semaphores, so
putting them on a separate ring keeps loads free of head-of-line blocking);
the low_out mask chain runs on the vector engine (DVE) as two fused
scalar_tensor_tensor ops + one multiply. Measured ~160 us/pass per core
(~300 GB/s/core; 48 MiB traffic vs the ~358 GB/s HBM-per-core roofline).
"""

import numpy as np

N_TOTAL = 16777216
N_CORES = 8
N_CORE = N_TOTAL // N_CORES  # 2097152
P = 128
FD = 2048      # free-dim elements per tile -> [128, 2048] f32 = 1 MiB per DMA
BUFS = 4       # io tile double-buffering depth
SCR_BUFS = 2   # scratch (mask) tile buffers

_CACHE = {}


def _build_nc(repeat=1, fd=FD, bufs=BUFS, scr_bufs=SCR_BUFS):
    from concourse import bacc, mybir
    from concourse.tile import TileContext

    f32 = mybir.dt.float32
    Alu = mybir.AluOpType
    Act = mybir.ActivationFunctionType

    # Bacc (not raw Bass): its compile() pass splits excess semaphore waits
    # onto InstEventSemaphore instructions (TRN2 allows 1 fused wait/inst).
    nc = bacc.Bacc(None, target_bir_lowering=False)
    x = nc.dram_tensor("x", [N_CORE], f32, kind="ExternalInput")
    low = nc.dram_tensor("low", [N_CORE], f32, kind="ExternalInput")
    high = nc.dram_tensor("high", [N_CORE], f32, kind="ExternalInput")
    x_out = nc.dram_tensor("x_out", [N_CORE], f32, kind="ExternalOutput")
    low_out = nc.dram_tensor("low_out", [N_CORE], f32, kind="ExternalOutput")
    high_out = nc.dram_tensor("high_out", [N_CORE], f32, kind="ExternalOutput")

    def tiled(t):
        return t.rearrange("(n p m) -> n p m", p=P, m=fd)

    xr, lr, hr = tiled(x), tiled(low), tiled(high)
    xor_, lor_, hor_ = tiled(x_out), tiled(low_out), tiled(high_out)
    ntiles = N_CORE // (P * fd)

    with TileContext(nc) as tc:
        with tc.tile_pool(name="pool", bufs=bufs) as pool, \
             tc.tile_pool(name="scrp", bufs=scr_bufs) as scrp:
            for i in [i for _ in range(repeat) for i in range(ntiles)]:
                xt = pool.tile([P, fd], f32)
                lt = pool.tile([P, fd], f32)
                ht = pool.tile([P, fd], f32)
                k1 = scrp.tile([P, fd], f32, tag="k1")
                tt = scrp.tile([P, fd], f32, tag="tt")

                # loads on the SP HWDGE ring
                nc.sync.dma_start(out=xt[:, :], in_=xr[i, :, :])
                nc.sync.dma_start(out=lt[:, :], in_=lr[i, :, :])
                nc.sync.dma_start(out=ht[:, :], in_=hr[i, :, :])

                # k1 = [(-low) <= high]  (lambda keep-mask)
                nc.vector.scalar_tensor_tensor(
                    out=k1[:, :], in0=lt[:, :], scalar=-1.0, in1=ht[:, :],
                    op0=Alu.mult, op1=Alu.is_le,
                )
                # tt = [high > 0] * low  (zero the inactive case)
                nc.vector.scalar_tensor_tensor(
                    out=tt[:, :], in0=ht[:, :], scalar=0.0, in1=lt[:, :],
                    op0=Alu.is_gt, op1=Alu.mult,
                )
                # low_out = k1 * tt   (in-place into the low tile)
                nc.vector.tensor_mul(out=lt[:, :], in0=k1[:, :], in1=tt[:, :])

                # relus in place on ACT (scheduled after the ht reads above)
                nc.scalar.activation(out=xt[:, :], in_=xt[:, :], func=Act.Relu)
                nc.scalar.activation(out=ht[:, :], in_=ht[:, :], func=Act.Relu)

                # stores on the ACT HWDGE ring
                nc.scalar.dma_start(out=xor_[i, :, :], in_=xt[:, :])
                nc.scalar.dma_start(out=lor_[i, :, :], in_=lt[:, :])
                nc.scalar.dma_start(out=hor_[i, :, :], in_=ht[:, :])
    nc.finalize()  # runs Bacc.compile(): wait-splitting, reg alloc, DCE
    return nc


def _get_nc():
    if "nc" not in _CACHE:
        _CACHE["nc"] = _build_nc()
    return _CACHE["nc"]


def kernel(x, low, high):
    from concourse.bass_utils import run_bass_kernel_spmd

    x = np.ascontiguousarray(np.asarray(x, dtype=np.float32).reshape(-1))
    low = np.ascontiguousarray(np.asarray(low, dtype=np.float32).reshape(-1))
    high = np.ascontiguousarray(np.asarray(high, dtype=np.float32).reshape(-1))
    assert x.shape == (N_TOTAL,)

    in_maps = []
    for c in range(N_CORES):
        sl = slice(c * N_CORE, (c + 1) * N_CORE)
        in_maps.append({
            "x": np.ascontiguousarray(x[sl]),
            "low": np.ascontiguousarray(low[sl]),
            "high": np.ascontiguousarray(high[sl]),
        })

    nc = _get_nc()
    res = run_bass_kernel_spmd(nc, in_maps, core_ids=list(range(N_CORES)))

    x_out = np.concatenate([res.results[c]["x_out"] for c in range(N_CORES)])
    low_out = np.concatenate([res.results[c]["low_out"] for c in range(N_CORES)])
    high_out = np.concatenate([res.results[c]["high_out"] for c in range(N_CORES)])
    return np.stack([x_out, low_out, high_out])
